# revision 39
# baseline (speedup 1.0000x reference)
"""Trainium2 Bass kernel for nn_ItemVectorTransform.

out = concat([x, softmax(x @ M.T) @ M], -1)   x:[2048,50] f32, M:[100000,50] f32

Strategy: data-parallel over batch B across 8 cores (256 rows each), memory
bank M replicated. Per core, a flash-style streaming pass over K in chunks of
128 rows with a no-max softmax (fixed bias -25 instead of a running max):

  mm1: sT'[k,b] = (A*M|B)_chunk @ (x|1)^T  (fp16, f32 PSUM) -- the Schraudolph
       exp affine i16 = A*s + B is folded into the matmul via an extra
       ones-row, and the 51-row contraction is zero-padded to 128 rows
  exp: pT[k,b] = bf16-bitcast(i16(sT'))    ~= exp(s - 25) * (1 +- 2%)
       (plain f32->i16 convert-copies, split across ACT and DVE)
  mm2: acc[b,d'] += pT_half^T @ [M|1]_chunk   (bf16, accumulated in PSUM)

The kernel is PE-bound at 2.4 GHz: ~84us of mm1 streams (784 x 256 cols, the
128-lane output floor) + ~42us of mm2 (1568 LDW/MM pairs).  Hard-won rules:

  - HAM clock gate: only FULL-ARRAY matmuls count as PE activity.  A 50-row
    row-group mm1 lets the 3.4us MID window expire and demotes the PE to
    1.2 GHz for the whole main loop (the original version ran that way:
    256-col mm1 at 213ns = 256cyc/1.2GHz).  Padding the contraction to 128
    rows (zero rows in the moving xt, memset-zeroed rows in the stationary
    ring slots -- junk can be NaN and 0*NaN=NaN) keeps 2.4 GHz and also
    qualifies mm1's LDWEIGHTS for FWL (~53ns instead of ~147ns).
  - sT lives in per-window PSUM tiles (7 banks; hazard tracking is
    tile-granular, so windows = tiles); acc is the 8th bank ([128, 2*51]).
  - exp work is divided per 14-chunk ring revolution into 4 column windows
    handled by ACT / DVE / ACT / DVE (GPSIMD cannot read PSUM); the 2048/1536
    col split matches their 1.2/0.96 GHz clocks.  The bitcast trick's +-2%
    multiplicative noise washes out in the softmax ratio (rel err ~4e-3).
  - pT lives in per-window bf16 SBUF tiles, 4 revolutions deep; mm2 runs
    >=20 chunks behind mm1 and is emitted in 12-chunk bursts (small bursts
    dovetail the LDWEIGHTS pipe with mm1's stream-bound stretches best).
  - DMA: ~21MB/core must stream through queue engines at ~35GB/s each, so
    layout and queue spread dominate: mn is packed 4 groups per transfer
    (one contiguous 6.5KB run per partition, 4x fewer descriptors), and
    transfers alternate between the SP and ACT HWDGE queues (each gets its
    own set of DMA engines; sync's Q_I alone is sometimes served by only 3).
    mt groups are split 2-way (4-way for the startup-critical first three)
    so each group streams on two queues concurrently, and prefetch runs 4
    groups / 3 blocks deep to ride out slow-DMA periods.

Epilogue (division by the ones-column denominator + concat) is on host.
"""

import os
import sys

for _p in ("/opt/trn_rl_repo", "/root/.axon_site/_ro/trn_rl_repo"):
    if os.path.isdir(_p) and _p not in sys.path:
        sys.path.insert(0, _p)

import numpy as np
import ml_dtypes

import concourse.bacc as bacc
import concourse.mybir as mybir
from concourse import tile
from concourse.bass_utils import run_bass_kernel_spmd

B, K, D = 2048, 100000, 50
N_CORES = 8
BC = B // N_CORES          # 256 batch rows per core
CHUNK = 128                # k rows per matmul chunk
GROUP = 16                 # chunks per DMA group
KP = 100352                # 49 * 2048, zero-padded K
NG = KP // (CHUNK * GROUP) # 49 DMA groups
NCHUNK = KP // CHUNK       # 784 chunks
DP1 = D + 1                # 51 (M columns + ones column)
MNB = 4                    # mn DMA block: groups per transfer.  Each DMA
                           # descriptor covers one partition's contiguous run;
                           # per-group mn transfers cost 128 descriptors for
                           # only 1.6KB each (~650us of total DMA queue time,
                           # saturating the engines) -- 4-group blocks move
                           # the same bytes in 1/4 the descriptors.
NB = (NG + MNB - 1) // MNB # 13 mn blocks (last one padded)
EXP_BIAS = -25.0

RING = 14                  # sT ring depth in chunks (14*256 f32 = 7 PSUM banks)
PT_REVS = 4                # pT ring depth in revolutions (4*14 chunks)
MM2_LAG = 20               # min chunks between mm1 and the consuming mm2
MM2_BATCH = 12             # mm2 chunks emitted per burst.  The first mm1
                           # after a burst pays an LDWEIGHTS-pipe backlog
                           # roughly proportional to burst length (mm2 LDW
                           # 45ns > mm2 MM 26ns), so smaller bursts dovetail
                           # better: measured +3.1us total mm1 stall at 12 vs
                           # +6.6us at 24.  lag+batch must stay under the pT
                           # ring depth (4*14=56).
# No dedicated warm-up burst: mm1 is now a full-array 128-row matmul, so the
# first ~3.4us of real chunks ramp the HAM clock themselves (at 1.2 GHz) and
# everything after runs at 2.4 GHz.  A warm-up burst would only delay the
# first chunk.
WARMUP_MM = 0

LOG2E = 1.4426950408889634
A16 = 128.0 * LOG2E                      # Schraudolph slope for bf16-bitcast
SIGMA = 0.05340                          # calibrated: zero mean log-error
B16 = 128.0 * (127.0 - SIGMA) + EXP_BIAS * A16
DR = D + 1                               # mt rows: D scaled-M rows + B16 row

# Per-revolution exp window grid: (start_col, end_col, engine).  Each window
# is its own PSUM tile (hazard tracking is tile-granular) so sizes must be
# whole PSUM banks (multiples of 512 f32): 2+2+1+2 banks here, acc is the
# 8th.  GPSIMD cannot read PSUM (BIR verifier), so ACT + DVE share the exp
# work; both are kept at <=80% so window reads finish with >100ns of margin
# (late window reads stall the PE past the clock-demotion threshold).
WINDOWS = [
    (0, 1024, "A"),
    (1024, 2048, "D"),
    (2048, 3072, "A"),
    (3072, 3584, "D"),
]

_nc_cache = None


def _install_trace_support():
    """The container's antenv lacks axon_hooks; synthesize it from trn_boot's
    ctypes NTFF shim so run_bass_kernel_spmd(trace=True) can profile."""
    import types

    if "antenv.axon_hooks" not in sys.modules:
        bootdir = "/root/.axon_site/trn_agent_boot"
        if bootdir not in sys.path:
            sys.path.insert(0, bootdir)
        import trn_boot

        hook = trn_boot._ntff_profile_via_ctypes("/opt/axon/libaxon_pjrt.so")
        mod = types.ModuleType("antenv.axon_hooks")
        mod.get_axon_ntff_profile_hook = lambda: hook
        mod.set_axon_ntff_profile_hook = lambda h: None
        sys.modules["antenv.axon_hooks"] = mod

    # No artifact bucket in this container; keep the NEFF dir local.
    import concourse.bass_utils as bu

    bu.upload_artifacts = lambda tmpdir: tmpdir


def _build():
    fp16 = mybir.dt.float16
    bf16 = mybir.dt.bfloat16
    f32 = mybir.dt.float32
    i16 = mybir.dt.int16

    nc = bacc.Bacc("TRN2", debug=False, num_devices=N_CORES)
    # xt is padded to 128 contraction rows (rows DR..127 zero) so mm1 can be a
    # FULL-ARRAY matmul: partial row-group MMs (50 rows) don't count as PE
    # activity for the HAM clock gate, which re-throttled the whole main loop
    # to 1.2 GHz in the previous version.  Full 128-row weights also enable
    # FWL (fast weight load), halving mm1 LDWEIGHTS to ~53ns.
    #
    # The Schraudolph exp affine (i16 = A16*s + B16) is folded INTO mm1: the
    # host pre-scales M by A16 and row D of the stationary holds B16, paired
    # with a ones-row in xt.  Both exp engines then run plain f32->i16
    # convert-copies (ACT: no spline table, cheaper op class).  The fp16
    # rounding of B16 is a GLOBAL factor on all exp values and cancels in the
    # softmax ratio.
    xt_d = nc.dram_tensor("xt", [CHUNK, BC], fp16, kind="ExternalInput")
    mtp_d = nc.dram_tensor("mtp", [DR, KP], fp16, kind="ExternalInput")
    mnp_d = nc.dram_tensor(
        "mnp", [NB, CHUNK, MNB * GROUP * DP1], bf16, kind="ExternalInput"
    )
    out_d = nc.dram_tensor("outU", [CHUNK, 2 * DP1], f32, kind="ExternalOutput")

    # window end -> (start, engine), keyed by the in-revolution chunk index
    # whose mm1 completes the window
    trig = {(we + BC - 1) // BC - 1: (ws, we, eng) for ws, we, eng in WINDOWS}

    with tile.TileContext(nc) as tc:
        with (
            tc.tile_pool(name="const", bufs=1) as constp,
            tc.tile_pool(name="mt", bufs=8) as mt_pool,
            tc.tile_pool(name="mn", bufs=6) as mn_pool,
            tc.tile_pool(name="wt0", bufs=PT_REVS) as wt0_pool,
            tc.tile_pool(name="wt1", bufs=PT_REVS) as wt1_pool,
            tc.tile_pool(name="wt2", bufs=PT_REVS) as wt2_pool,
            tc.tile_pool(name="wt3", bufs=PT_REVS) as wt3_pool,
            tc.tile_pool(name="wt4", bufs=PT_REVS) as wt4_pool,
            tc.tile_pool(name="ps0", bufs=1, space="PSUM") as ps0_pool,
            tc.tile_pool(name="ps1", bufs=1, space="PSUM") as ps1_pool,
            tc.tile_pool(name="ps2", bufs=1, space="PSUM") as ps2_pool,
            tc.tile_pool(name="ps3", bufs=1, space="PSUM") as ps3_pool,
            tc.tile_pool(name="ps4", bufs=1, space="PSUM") as ps4_pool,
            tc.tile_pool(name="acc", bufs=1, space="PSUM") as acc_pool,
        ):
            wpools = [wt0_pool, wt1_pool, wt2_pool, wt3_pool, wt4_pool]
            pspools = [ps0_pool, ps1_pool, ps2_pool, ps3_pool, ps4_pool]
            xt = constp.tile([CHUNK, BC], fp16)
            nc.sync.dma_start(out=xt[:], in_=xt_d[:])
            bias = constp.tile([CHUNK, 1], f32)
            nc.vector.memset(bias[:], EXP_BIAS)
            dummy = constp.tile([CHUNK, 1], bf16)
            # front-load any ACT table load for the Copy func during input DMA
            nc.scalar.copy(dummy[:].bitcast(i16), bias[:])

            # per-window sT tiles in PSUM (7 banks total), single-buffered:
            # the "ring" reuse hazard is at window granularity
            spsum = [
                pspools[i].tile([CHUNK, we - ws], f32, name=f"spsum{i}")
                for i, (ws, we, _) in enumerate(WINDOWS)
            ]
            acc = acc_pool.tile([CHUNK, 2 * DP1], f32)       # 1 PSUM bank
            # per-window pT tiles (separate tiles so the engines' writes
            # carry no false hazards against each other); keyed by window
            # index, double-buffered across revolutions by the pools
            wtiles = {}

            # Pre-zero rows D..127 of the mt ring slots once: the stationary
            # operand is read as full [128, .] tiles (for FWL + full-array HAM
            # activity) but the DMA only fills rows 0..D-1; rows D..127 must be
            # 0.0, not SBUF junk (junk can hold Inf/NaN patterns and 0*Inf =
            # NaN).  Each ring position gets its own tag with bufs=1 so every
            # allocation of that tag reuses the same physical slot — the
            # scheduler's lifetime-based slot assignment would otherwise let a
            # never-memset slot slip in.
            for i in range(8):
                mtz = mt_pool.tile(
                    [CHUNK, CHUNK * GROUP], fp16, name="mtz", tag=f"mt{i}", bufs=1
                )
                # full-tile memset on the otherwise-idle GpSimd engine (a
                # row-sliced memset would need a 32-aligned base partition;
                # the group DMA overwrites rows 0..D-1 anyway)
                nc.gpsimd.memset(mtz[:], 0.0)

            mt_tiles = {}
            mn_blocks = {}

            CG = CHUNK * GROUP
            GW = GROUP * DP1

            def issue_group_dma(g):
                if g >= NG:
                    return
                # Each dma_start costs ~700ns of DIRECT2D descriptor work on
                # the Sync sequencer, so steady-state groups use ONE transfer
                # per tensor (a 4-way split everywhere put 296 DIRECT2Ds =
                # 218us on it and starved the PE).  Only the early groups are
                # split so the kernel's first chunks land sooner.
                mt = mt_pool.tile(
                    [CHUNK, CG], fp16, name="mt", tag=f"mt{g % 8}", bufs=1
                )
                # alternate between the two HWDGE queues (sync=Q_I,
                # scalar=Q_X): the runtime serves each queue with its own set
                # of DMA engines, and Q_I alone is sometimes given only 3
                eng = nc.sync if g % 2 == 0 else nc.scalar
                # every group split 2-way (4-way for the startup-critical
                # first three): halves per-group landing latency when the DMA
                # fabric runs slow, at ~65 DIRECT2Ds per queue (well under
                # the ~300 that starved the sync sequencer)
                nsplit = 4 if g < 3 else 2
                for s in range(nsplit):
                    w = CG // nsplit
                    eng.dma_start(
                        out=mt[:DR, s * w : (s + 1) * w],
                        in_=mtp_d[:, g * CG + s * w : g * CG + (s + 1) * w],
                    )
                mt_tiles[g] = mt

            def issue_block_dma(b):
                if b >= NB:
                    return
                # issue from the ACT engine's HWDGE queue: all sync-issued
                # DMAs share one hardware queue (Q_I) whose engine assignment
                # varies run to run (155us vs 219us!); a second queue doubles
                # the DMA engines serving the kernel's ~21MB
                mnb = mn_pool.tile([CHUNK, MNB * GW], bf16, name="mnb")
                half = MNB * GW // 2
                e0 = nc.scalar if b % 2 == 0 else nc.sync
                e1 = nc.sync if b % 2 == 0 else nc.scalar
                e0.dma_start(out=mnb[:, :half], in_=mnp_d[b][:, :half])
                e1.dma_start(out=mnb[:, half:], in_=mnp_d[b][:, half:])
                mn_blocks[b] = mnb

            for g in range(5):
                issue_group_dma(g)
            for b in range(4):
                issue_block_dma(b)

            # 128-col block index within a revolution -> window index
            win_of = [
                next(i for i, (ws, we, _) in enumerate(WINDOWS) if ws <= col < we)
                for col in range(0, RING * BC, CHUNK)
            ]

            def emit_mm2_batch(chunks):
                # acc[b, d'] += pT_half[k, b]^T @ [M|1]_chunk[k, d'] with the
                # exp output pT as the stationary operand.  The whole kernel
                # is ONE psum accumulation group in the acc bank; emitting the
                # batch as [all h0, then all h1] keeps region switches rare.
                for h in range(2):
                    for ch in chunks:
                        g, j = divmod(ch, GROUP)
                        bi, gi = divmod(g, MNB)
                        mnb = mn_blocks[bi]
                        par = (ch // RING) % PT_REVS
                        revcol = (ch % RING) * BC + h * CHUNK
                        wi = win_of[revcol // CHUNK]
                        off = revcol - WINDOWS[wi][0]
                        c0 = (gi * GROUP + j) * DP1
                        nc.tensor.matmul(
                            acc[:, h * DP1 : (h + 1) * DP1],
                            wtiles[par, wi][:, off : off + CHUNK],
                            mnb[:, c0 : c0 + DP1],
                            start=(ch == 0 and h == 0),
                            stop=(ch == NCHUNK - 1 and h == 1),
                        )
                for ch in chunks:
                    g, j = divmod(ch, GROUP)
                    if j == GROUP - 1 and (g % MNB == MNB - 1 or g == NG - 1):
                        mn_blocks.pop(g // MNB)

            next_mm2 = 0
            for c in range(NCHUNK):
                g, j = divmod(c, GROUP)
                if j == 0:
                    issue_group_dma(g + 5)
                if c % (MNB * GROUP) == 0:
                    issue_block_dma(c // (MNB * GROUP) + 4)
                rev, rc = divmod(c, RING)
                if c - MM2_LAG - next_mm2 + 1 >= MM2_BATCH:
                    emit_mm2_batch(range(next_mm2, next_mm2 + MM2_BATCH))
                    next_mm2 += MM2_BATCH
                revcol = rc * BC
                mwi = win_of[revcol // CHUNK]
                moff = revcol - WINDOWS[mwi][0]
                nc.tensor.matmul(
                    spsum[mwi][:, moff : moff + BC],
                    mt_tiles[g][:, j * CHUNK : (j + 1) * CHUNK],
                    xt[:],
                    start=True,
                    stop=True,
                )
                if j == GROUP - 1:
                    mt_tiles.pop(g)
                if rc in trig:
                    ws, we, eng = trig[rc]
                    wi = WINDOWS.index((ws, we, eng))
                    wt = wpools[wi].tile([CHUNK, we - ws], bf16, name=f"wt{wi}")
                    wtiles[rev % PT_REVS, wi] = wt
                    src = spsum[wi][:]
                    # mm1 already produced A16*s + B16; the exp is now just a
                    # rounding f32->i16 convert whose bitcast-to-bf16 is
                    # exp(s-25) to within the usual Schraudolph +-2%
                    if eng == "A":
                        nc.scalar.copy(wt[:].bitcast(i16), src)
                    else:
                        nc.vector.tensor_copy(wt[:].bitcast(i16), src)

            while next_mm2 < NCHUNK:
                hi = min(next_mm2 + MM2_BATCH, NCHUNK)
                emit_mm2_batch(range(next_mm2, hi))
                next_mm2 = hi

            out_sb = constp.tile([CHUNK, 2 * DP1], f32)
            nc.vector.tensor_copy(out_sb[:], acc[:])
            nc.sync.dma_start(out=out_d[:], in_=out_sb[:])

    nc.compile()
    return nc


def _get_nc():
    global _nc_cache
    if _nc_cache is None:
        _nc_cache = _build()
    return _nc_cache


def _prep_inputs(x, M):
    x = np.asarray(x, dtype=np.float32)
    M = np.asarray(M, dtype=np.float32)

    mtp = np.zeros((DR, KP), dtype=np.float16)
    mtp[:D, :K] = (A16 * M.T).astype(np.float16)
    mtp[D, :] = np.float16(B16)

    mn = np.zeros((KP, DP1), dtype=np.float32)
    mn[:K, :D] = M
    mn[:, D] = 1.0
    # [g, j, p, d] -> [g, p, j*51+d] so each partition's row is contiguous
    mnp1 = np.ascontiguousarray(
        mn.reshape(NG, GROUP, CHUNK, DP1).transpose(0, 2, 1, 3)
    ).reshape(NG, CHUNK, GROUP * DP1)
    # pack MNB groups per DMA block: [b, p, gi*816 + j*51 + d], one contiguous
    # per-partition run per block (4x fewer DMA descriptors)
    mnp_pad = np.zeros((NB * MNB, CHUNK, GROUP * DP1), dtype=np.float32)
    mnp_pad[:NG] = mnp1
    mnp = np.ascontiguousarray(
        mnp_pad.reshape(NB, MNB, CHUNK, GROUP * DP1).transpose(0, 2, 1, 3)
    ).reshape(NB, CHUNK, MNB * GROUP * DP1).astype(ml_dtypes.bfloat16)

    in_maps = []
    for i in range(N_CORES):
        xt = np.zeros((CHUNK, BC), dtype=np.float16)
        xt[:D] = x[i * BC : (i + 1) * BC].T.astype(np.float16)
        xt[D] = np.float16(1.0)  # pairs with the B16 row of the stationary
        in_maps.append({"xt": xt, "mtp": mtp, "mnp": mnp})
    return in_maps


def _run(x, M, trace=False):
    if trace:
        _install_trace_support()
    nc = _get_nc()
    in_maps = _prep_inputs(x, M)
    res = run_bass_kernel_spmd(nc, in_maps, core_ids=list(range(N_CORES)), trace=trace)
    x = np.asarray(x, dtype=np.float32)
    u = np.empty((B, D), dtype=np.float32)
    for i in range(N_CORES):
        raw = res.results[i]["outU"]  # [128, 2*51] — per-half accumulators
        for h in range(2):
            seg = raw[:, h * DP1 : (h + 1) * DP1]  # [128, 51] natural [b, d']
            r0 = i * BC + h * CHUNK
            u[r0 : r0 + CHUNK] = seg[:, :D] / seg[:, D : D + 1]
    out = np.concatenate([x, u], axis=1)
    return out, res


def kernel(x, M):
    out, _ = _run(x, M, trace=False)
    return out



# revision 40
# speedup vs baseline: 1.0404x; 1.0404x over previous
"""Trainium2 Bass kernel for nn_ItemVectorTransform.

out = concat([x, softmax(x @ M.T) @ M], -1)   x:[2048,50] f32, M:[100000,50] f32

Strategy: data-parallel over batch B across 8 cores (256 rows each), memory
bank M replicated. Per core, a flash-style streaming pass over K in chunks of
128 rows with a no-max softmax (fixed bias -25 instead of a running max):

  mm1: sT'[k,b] = (A*M|B)_chunk @ (x|1)^T  (fp16, f32 PSUM) -- the Schraudolph
       exp affine i16 = A*s + B is folded into the matmul via an extra
       ones-row, and the 51-row contraction is zero-padded to 128 rows
  exp: pT[k,b] = bf16-bitcast(i16(sT'))    ~= exp(s - 25) * (1 +- 2%)
       (plain f32->i16 convert-copies, split across ACT and DVE)
  mm2: acc[b,d'] += pT_half^T @ [M|1]_chunk   (bf16, accumulated in PSUM)

The kernel is PE-bound at 2.4 GHz: ~84us of mm1 streams (784 x 256 cols, the
128-lane output floor) + ~42us of mm2 (1568 LDW/MM pairs).  Hard-won rules:

  - HAM clock gate: only FULL-ARRAY matmuls count as PE activity.  A 50-row
    row-group mm1 lets the 3.4us MID window expire and demotes the PE to
    1.2 GHz for the whole main loop (the original version ran that way:
    256-col mm1 at 213ns = 256cyc/1.2GHz).  Padding the contraction to 128
    rows (zero rows in the moving xt, memset-zeroed rows in the stationary
    ring slots -- junk can be NaN and 0*NaN=NaN) keeps 2.4 GHz and also
    qualifies mm1's LDWEIGHTS for FWL (~53ns instead of ~147ns).
  - sT lives in per-window PSUM tiles (7 banks; hazard tracking is
    tile-granular, so windows = tiles); acc is the 8th bank ([128, 2*51]).
  - exp work is divided per 14-chunk ring revolution into 4 column windows
    handled by ACT / DVE / ACT / DVE (GPSIMD cannot read PSUM); the 2048/1536
    col split matches their 1.2/0.96 GHz clocks.  The bitcast trick's +-2%
    multiplicative noise washes out in the softmax ratio (rel err ~4e-3).
  - pT lives in per-window bf16 SBUF tiles, 4 revolutions deep; mm2 runs
    >=20 chunks behind mm1 and is emitted in 12-chunk bursts (small bursts
    dovetail the LDWEIGHTS pipe with mm1's stream-bound stretches best).
  - DMA: ~21MB/core must stream through queue engines at ~35GB/s each, so
    layout and queue spread dominate: mn is packed 4 groups per transfer
    (one contiguous 6.5KB run per partition, 4x fewer descriptors), and
    transfers alternate between the SP and ACT HWDGE queues (each gets its
    own set of DMA engines; sync's Q_I alone is sometimes served by only 3).
    mt groups are split 4/2-way early so the first chunks land sooner, and
    prefetch runs 4 groups / 3 blocks deep to ride out slow-DMA periods.

Epilogue (division by the ones-column denominator + concat) is on host.
"""

import os
import sys

for _p in ("/opt/trn_rl_repo", "/root/.axon_site/_ro/trn_rl_repo"):
    if os.path.isdir(_p) and _p not in sys.path:
        sys.path.insert(0, _p)

import numpy as np
import ml_dtypes

import concourse.bacc as bacc
import concourse.mybir as mybir
from concourse import tile
from concourse.bass_utils import run_bass_kernel_spmd

B, K, D = 2048, 100000, 50
N_CORES = 8
BC = B // N_CORES          # 256 batch rows per core
CHUNK = 128                # k rows per matmul chunk
GROUP = 16                 # chunks per DMA group
KP = 100352                # 49 * 2048, zero-padded K
NG = KP // (CHUNK * GROUP) # 49 DMA groups
NCHUNK = KP // CHUNK       # 784 chunks
DP1 = D + 1                # 51 (M columns + ones column)
MNB = 4                    # mn DMA block: groups per transfer.  Each DMA
                           # descriptor covers one partition's contiguous run;
                           # per-group mn transfers cost 128 descriptors for
                           # only 1.6KB each (~650us of total DMA queue time,
                           # saturating the engines) -- 4-group blocks move
                           # the same bytes in 1/4 the descriptors.
NB = (NG + MNB - 1) // MNB # 13 mn blocks (last one padded)
EXP_BIAS = -25.0

RING = 14                  # sT ring depth in chunks (14*256 f32 = 7 PSUM banks)
PT_REVS = 4                # pT ring depth in revolutions (4*14 chunks)
MM2_LAG = 20               # min chunks between mm1 and the consuming mm2
MM2_BATCH = 12             # mm2 chunks emitted per burst.  The first mm1
                           # after a burst pays an LDWEIGHTS-pipe backlog
                           # roughly proportional to burst length (mm2 LDW
                           # 45ns > mm2 MM 26ns), so smaller bursts dovetail
                           # better: measured +3.1us total mm1 stall at 12 vs
                           # +6.6us at 24.  lag+batch must stay under the pT
                           # ring depth (4*14=56).
# No dedicated warm-up burst: mm1 is now a full-array 128-row matmul, so the
# first ~3.4us of real chunks ramp the HAM clock themselves (at 1.2 GHz) and
# everything after runs at 2.4 GHz.  A warm-up burst would only delay the
# first chunk.
WARMUP_MM = 0

LOG2E = 1.4426950408889634
A16 = 128.0 * LOG2E                      # Schraudolph slope for bf16-bitcast
SIGMA = 0.05340                          # calibrated: zero mean log-error
B16 = 128.0 * (127.0 - SIGMA) + EXP_BIAS * A16
DR = D + 1                               # mt rows: D scaled-M rows + B16 row

# Per-revolution exp window grid: (start_col, end_col, engine).  Each window
# is its own PSUM tile (hazard tracking is tile-granular) so sizes must be
# whole PSUM banks (multiples of 512 f32): 2+2+1+2 banks here, acc is the
# 8th.  GPSIMD cannot read PSUM (BIR verifier), so ACT + DVE share the exp
# work; both are kept at <=80% so window reads finish with >100ns of margin
# (late window reads stall the PE past the clock-demotion threshold).
WINDOWS = [
    (0, 1024, "A"),
    (1024, 2048, "D"),
    (2048, 3072, "A"),
    (3072, 3584, "D"),
]

_nc_cache = None


def _install_trace_support():
    """The container's antenv lacks axon_hooks; synthesize it from trn_boot's
    ctypes NTFF shim so run_bass_kernel_spmd(trace=True) can profile."""
    import types

    if "antenv.axon_hooks" not in sys.modules:
        bootdir = "/root/.axon_site/trn_agent_boot"
        if bootdir not in sys.path:
            sys.path.insert(0, bootdir)
        import trn_boot

        hook = trn_boot._ntff_profile_via_ctypes("/opt/axon/libaxon_pjrt.so")
        mod = types.ModuleType("antenv.axon_hooks")
        mod.get_axon_ntff_profile_hook = lambda: hook
        mod.set_axon_ntff_profile_hook = lambda h: None
        sys.modules["antenv.axon_hooks"] = mod

    # No artifact bucket in this container; keep the NEFF dir local.
    import concourse.bass_utils as bu

    bu.upload_artifacts = lambda tmpdir: tmpdir


def _build():
    fp16 = mybir.dt.float16
    bf16 = mybir.dt.bfloat16
    f32 = mybir.dt.float32
    i16 = mybir.dt.int16

    nc = bacc.Bacc("TRN2", debug=False, num_devices=N_CORES)
    # xt is padded to 128 contraction rows (rows DR..127 zero) so mm1 can be a
    # FULL-ARRAY matmul: partial row-group MMs (50 rows) don't count as PE
    # activity for the HAM clock gate, which re-throttled the whole main loop
    # to 1.2 GHz in the previous version.  Full 128-row weights also enable
    # FWL (fast weight load), halving mm1 LDWEIGHTS to ~53ns.
    #
    # The Schraudolph exp affine (i16 = A16*s + B16) is folded INTO mm1: the
    # host pre-scales M by A16 and row D of the stationary holds B16, paired
    # with a ones-row in xt.  Both exp engines then run plain f32->i16
    # convert-copies (ACT: no spline table, cheaper op class).  The fp16
    # rounding of B16 is a GLOBAL factor on all exp values and cancels in the
    # softmax ratio.
    xt_d = nc.dram_tensor("xt", [CHUNK, BC], fp16, kind="ExternalInput")
    mtp_d = nc.dram_tensor("mtp", [DR, KP], fp16, kind="ExternalInput")
    mnp_d = nc.dram_tensor(
        "mnp", [NB, CHUNK, MNB * GROUP * DP1], bf16, kind="ExternalInput"
    )
    out_d = nc.dram_tensor("outU", [CHUNK, 2 * DP1], f32, kind="ExternalOutput")

    # window end -> (start, engine), keyed by the in-revolution chunk index
    # whose mm1 completes the window
    trig = {(we + BC - 1) // BC - 1: (ws, we, eng) for ws, we, eng in WINDOWS}

    with tile.TileContext(nc) as tc:
        with (
            tc.tile_pool(name="const", bufs=1) as constp,
            tc.tile_pool(name="mt", bufs=7) as mt_pool,
            tc.tile_pool(name="mn", bufs=5) as mn_pool,
            tc.tile_pool(name="wt0", bufs=PT_REVS) as wt0_pool,
            tc.tile_pool(name="wt1", bufs=PT_REVS) as wt1_pool,
            tc.tile_pool(name="wt2", bufs=PT_REVS) as wt2_pool,
            tc.tile_pool(name="wt3", bufs=PT_REVS) as wt3_pool,
            tc.tile_pool(name="wt4", bufs=PT_REVS) as wt4_pool,
            tc.tile_pool(name="ps0", bufs=1, space="PSUM") as ps0_pool,
            tc.tile_pool(name="ps1", bufs=1, space="PSUM") as ps1_pool,
            tc.tile_pool(name="ps2", bufs=1, space="PSUM") as ps2_pool,
            tc.tile_pool(name="ps3", bufs=1, space="PSUM") as ps3_pool,
            tc.tile_pool(name="ps4", bufs=1, space="PSUM") as ps4_pool,
            tc.tile_pool(name="acc", bufs=1, space="PSUM") as acc_pool,
        ):
            wpools = [wt0_pool, wt1_pool, wt2_pool, wt3_pool, wt4_pool]
            pspools = [ps0_pool, ps1_pool, ps2_pool, ps3_pool, ps4_pool]
            xt = constp.tile([CHUNK, BC], fp16)
            nc.sync.dma_start(out=xt[:], in_=xt_d[:])
            bias = constp.tile([CHUNK, 1], f32)
            nc.vector.memset(bias[:], EXP_BIAS)
            dummy = constp.tile([CHUNK, 1], bf16)
            # front-load any ACT table load for the Copy func during input DMA
            nc.scalar.copy(dummy[:].bitcast(i16), bias[:])

            # per-window sT tiles in PSUM (7 banks total), single-buffered:
            # the "ring" reuse hazard is at window granularity
            spsum = [
                pspools[i].tile([CHUNK, we - ws], f32, name=f"spsum{i}")
                for i, (ws, we, _) in enumerate(WINDOWS)
            ]
            acc = acc_pool.tile([CHUNK, 2 * DP1], f32)       # 1 PSUM bank
            # per-window pT tiles (separate tiles so the engines' writes
            # carry no false hazards against each other); keyed by window
            # index, double-buffered across revolutions by the pools
            wtiles = {}

            # Pre-zero rows D..127 of the mt ring slots once: the stationary
            # operand is read as full [128, .] tiles (for FWL + full-array HAM
            # activity) but the DMA only fills rows 0..D-1; rows D..127 must be
            # 0.0, not SBUF junk (junk can hold Inf/NaN patterns and 0*Inf =
            # NaN).  Each ring position gets its own tag with bufs=1 so every
            # allocation of that tag reuses the same physical slot — the
            # scheduler's lifetime-based slot assignment would otherwise let a
            # never-memset slot slip in.
            for i in range(7):
                mtz = mt_pool.tile(
                    [CHUNK, CHUNK * GROUP], fp16, name="mtz", tag=f"mt{i}", bufs=1
                )
                # full-tile memset on the otherwise-idle GpSimd engine (a
                # row-sliced memset would need a 32-aligned base partition;
                # the group DMA overwrites rows 0..D-1 anyway)
                nc.gpsimd.memset(mtz[:], 0.0)

            mt_tiles = {}
            mn_blocks = {}

            CG = CHUNK * GROUP
            GW = GROUP * DP1

            def issue_group_dma(g):
                if g >= NG:
                    return
                # Each dma_start costs ~700ns of DIRECT2D descriptor work on
                # the Sync sequencer, so steady-state groups use ONE transfer
                # per tensor (a 4-way split everywhere put 296 DIRECT2Ds =
                # 218us on it and starved the PE).  Only the early groups are
                # split so the kernel's first chunks land sooner.
                mt = mt_pool.tile(
                    [CHUNK, CG], fp16, name="mt", tag=f"mt{g % 7}", bufs=1
                )
                # alternate between the two HWDGE queues (sync=Q_I,
                # scalar=Q_X): the runtime serves each queue with its own set
                # of DMA engines, and Q_I alone is sometimes given only 3
                eng = nc.sync if g % 2 == 0 else nc.scalar
                # every group split 2-way (4-way for the startup-critical
                # first three): halves per-group landing latency when the DMA
                # fabric runs slow, at ~65 DIRECT2Ds per queue (well under
                # the ~300 that starved the sync sequencer)
                nsplit = 4 if g < 3 else 2
                for s in range(nsplit):
                    w = CG // nsplit
                    eng.dma_start(
                        out=mt[:DR, s * w : (s + 1) * w],
                        in_=mtp_d[:, g * CG + s * w : g * CG + (s + 1) * w],
                    )
                mt_tiles[g] = mt

            def issue_block_dma(b):
                if b >= NB:
                    return
                # issue from the ACT engine's HWDGE queue: all sync-issued
                # DMAs share one hardware queue (Q_I) whose engine assignment
                # varies run to run (155us vs 219us!); a second queue doubles
                # the DMA engines serving the kernel's ~21MB
                mnb = mn_pool.tile([CHUNK, MNB * GW], bf16, name="mnb")
                half = MNB * GW // 2
                eng = nc.scalar if b % 2 == 0 else nc.sync
                eng.dma_start(out=mnb[:, :half], in_=mnp_d[b][:, :half])
                eng.dma_start(out=mnb[:, half:], in_=mnp_d[b][:, half:])
                mn_blocks[b] = mnb

            for g in range(4):
                issue_group_dma(g)
            for b in range(3):
                issue_block_dma(b)

            # 128-col block index within a revolution -> window index
            win_of = [
                next(i for i, (ws, we, _) in enumerate(WINDOWS) if ws <= col < we)
                for col in range(0, RING * BC, CHUNK)
            ]

            def emit_mm2_batch(chunks):
                # acc[b, d'] += pT_half[k, b]^T @ [M|1]_chunk[k, d'] with the
                # exp output pT as the stationary operand.  The whole kernel
                # is ONE psum accumulation group in the acc bank; emitting the
                # batch as [all h0, then all h1] keeps region switches rare.
                for h in range(2):
                    for ch in chunks:
                        g, j = divmod(ch, GROUP)
                        bi, gi = divmod(g, MNB)
                        mnb = mn_blocks[bi]
                        par = (ch // RING) % PT_REVS
                        revcol = (ch % RING) * BC + h * CHUNK
                        wi = win_of[revcol // CHUNK]
                        off = revcol - WINDOWS[wi][0]
                        c0 = (gi * GROUP + j) * DP1
                        nc.tensor.matmul(
                            acc[:, h * DP1 : (h + 1) * DP1],
                            wtiles[par, wi][:, off : off + CHUNK],
                            mnb[:, c0 : c0 + DP1],
                            start=(ch == 0 and h == 0),
                            stop=(ch == NCHUNK - 1 and h == 1),
                        )
                for ch in chunks:
                    g, j = divmod(ch, GROUP)
                    if j == GROUP - 1 and (g % MNB == MNB - 1 or g == NG - 1):
                        mn_blocks.pop(g // MNB)

            next_mm2 = 0
            for c in range(NCHUNK):
                g, j = divmod(c, GROUP)
                if j == 0:
                    issue_group_dma(g + 4)
                if c % (MNB * GROUP) == 0:
                    issue_block_dma(c // (MNB * GROUP) + 3)
                rev, rc = divmod(c, RING)
                if c - MM2_LAG - next_mm2 + 1 >= MM2_BATCH:
                    emit_mm2_batch(range(next_mm2, next_mm2 + MM2_BATCH))
                    next_mm2 += MM2_BATCH
                revcol = rc * BC
                mwi = win_of[revcol // CHUNK]
                moff = revcol - WINDOWS[mwi][0]
                nc.tensor.matmul(
                    spsum[mwi][:, moff : moff + BC],
                    mt_tiles[g][:, j * CHUNK : (j + 1) * CHUNK],
                    xt[:],
                    start=True,
                    stop=True,
                )
                if j == GROUP - 1:
                    mt_tiles.pop(g)
                if rc in trig:
                    ws, we, eng = trig[rc]
                    wi = WINDOWS.index((ws, we, eng))
                    wt = wpools[wi].tile([CHUNK, we - ws], bf16, name=f"wt{wi}")
                    wtiles[rev % PT_REVS, wi] = wt
                    src = spsum[wi][:]
                    # mm1 already produced A16*s + B16; the exp is now just a
                    # rounding f32->i16 convert whose bitcast-to-bf16 is
                    # exp(s-25) to within the usual Schraudolph +-2%
                    if eng == "A":
                        nc.scalar.copy(wt[:].bitcast(i16), src)
                    else:
                        nc.vector.tensor_copy(wt[:].bitcast(i16), src)

            while next_mm2 < NCHUNK:
                hi = min(next_mm2 + MM2_BATCH, NCHUNK)
                emit_mm2_batch(range(next_mm2, hi))
                next_mm2 = hi

            out_sb = constp.tile([CHUNK, 2 * DP1], f32)
            nc.vector.tensor_copy(out_sb[:], acc[:])
            nc.sync.dma_start(out=out_d[:], in_=out_sb[:])

    nc.compile()
    return nc


def _get_nc():
    global _nc_cache
    if _nc_cache is None:
        _nc_cache = _build()
    return _nc_cache


def _prep_inputs(x, M):
    x = np.asarray(x, dtype=np.float32)
    M = np.asarray(M, dtype=np.float32)

    mtp = np.zeros((DR, KP), dtype=np.float16)
    mtp[:D, :K] = (A16 * M.T).astype(np.float16)
    mtp[D, :] = np.float16(B16)

    mn = np.zeros((KP, DP1), dtype=np.float32)
    mn[:K, :D] = M
    mn[:, D] = 1.0
    # [g, j, p, d] -> [g, p, j*51+d] so each partition's row is contiguous
    mnp1 = np.ascontiguousarray(
        mn.reshape(NG, GROUP, CHUNK, DP1).transpose(0, 2, 1, 3)
    ).reshape(NG, CHUNK, GROUP * DP1)
    # pack MNB groups per DMA block: [b, p, gi*816 + j*51 + d], one contiguous
    # per-partition run per block (4x fewer DMA descriptors)
    mnp_pad = np.zeros((NB * MNB, CHUNK, GROUP * DP1), dtype=np.float32)
    mnp_pad[:NG] = mnp1
    mnp = np.ascontiguousarray(
        mnp_pad.reshape(NB, MNB, CHUNK, GROUP * DP1).transpose(0, 2, 1, 3)
    ).reshape(NB, CHUNK, MNB * GROUP * DP1).astype(ml_dtypes.bfloat16)

    in_maps = []
    for i in range(N_CORES):
        xt = np.zeros((CHUNK, BC), dtype=np.float16)
        xt[:D] = x[i * BC : (i + 1) * BC].T.astype(np.float16)
        xt[D] = np.float16(1.0)  # pairs with the B16 row of the stationary
        in_maps.append({"xt": xt, "mtp": mtp, "mnp": mnp})
    return in_maps


def _run(x, M, trace=False):
    if trace:
        _install_trace_support()
    nc = _get_nc()
    in_maps = _prep_inputs(x, M)
    res = run_bass_kernel_spmd(nc, in_maps, core_ids=list(range(N_CORES)), trace=trace)
    x = np.asarray(x, dtype=np.float32)
    u = np.empty((B, D), dtype=np.float32)
    for i in range(N_CORES):
        raw = res.results[i]["outU"]  # [128, 2*51] — per-half accumulators
        for h in range(2):
            seg = raw[:, h * DP1 : (h + 1) * DP1]  # [128, 51] natural [b, d']
            r0 = i * BC + h * CHUNK
            u[r0 : r0 + CHUNK] = seg[:, :D] / seg[:, D : D + 1]
    out = np.concatenate([x, u], axis=1)
    return out, res


def kernel(x, M):
    out, _ = _run(x, M, trace=False)
    return out



# revision 41
# speedup vs baseline: 1.0453x; 1.0047x over previous
"""Trainium2 Bass kernel for nn_ItemVectorTransform.

out = concat([x, softmax(x @ M.T) @ M], -1)   x:[2048,50] f32, M:[100000,50] f32

Strategy: data-parallel over batch B across 8 cores (256 rows each), memory
bank M replicated. Per core, a flash-style streaming pass over K in chunks of
128 rows with a no-max softmax (fixed bias -25 instead of a running max):

  mm1: sT'[k,b] = (A*M|B)_chunk @ (x|1)^T  (fp16, f32 PSUM) -- the Schraudolph
       exp affine i16 = A*s + B is folded into the matmul via an extra
       ones-row, and the 51-row contraction is zero-padded to 128 rows
  exp: pT[k,b] = bf16-bitcast(i16(sT'))    ~= exp(s - 25) * (1 +- 2%)
       (plain f32->i16 convert-copies, split across ACT and DVE)
  mm2: acc[b,d'] += pT_half^T @ [M|1]_chunk   (bf16, accumulated in PSUM)

The kernel is PE-bound at 2.4 GHz: ~84us of mm1 streams (784 x 256 cols, the
128-lane output floor) + ~42us of mm2 (1568 LDW/MM pairs).  Hard-won rules:

  - HAM clock gate: only FULL-ARRAY matmuls count as PE activity.  A 50-row
    row-group mm1 lets the 3.4us MID window expire and demotes the PE to
    1.2 GHz for the whole main loop (the original version ran that way:
    256-col mm1 at 213ns = 256cyc/1.2GHz).  Padding the contraction to 128
    rows (zero rows in the moving xt, memset-zeroed rows in the stationary
    ring slots -- junk can be NaN and 0*NaN=NaN) keeps 2.4 GHz and also
    qualifies mm1's LDWEIGHTS for FWL (~53ns instead of ~147ns).
  - sT lives in per-window PSUM tiles (7 banks; hazard tracking is
    tile-granular, so windows = tiles); acc is the 8th bank ([128, 2*51]).
  - exp work is divided per 14-chunk ring revolution into 4 column windows
    handled by ACT / DVE / ACT / DVE (GPSIMD cannot read PSUM); the 2048/1536
    col split matches their 1.2/0.96 GHz clocks.  The bitcast trick's +-2%
    multiplicative noise washes out in the softmax ratio (rel err ~4e-3).
  - pT lives in per-window bf16 SBUF tiles, 4 revolutions deep; mm2 runs
    >=20 chunks behind mm1 and is emitted in 12-chunk bursts (small bursts
    dovetail the LDWEIGHTS pipe with mm1's stream-bound stretches best).
  - DMA: ~21MB/core must stream through queue engines at ~35GB/s each, so
    layout and queue spread dominate: mn is packed 4 groups per transfer
    (one contiguous 6.5KB run per partition, 4x fewer descriptors), and
    transfers alternate between the SP and ACT HWDGE queues (each gets its
    own set of DMA engines; sync's Q_I alone is sometimes served by only 3).
    mt groups are split 4/2-way early so the first chunks land sooner, and
    prefetch runs 4 groups / 3 blocks deep to ride out slow-DMA periods.

Epilogue (division by the ones-column denominator + concat) is on host.
"""

import os
import sys

for _p in ("/opt/trn_rl_repo", "/root/.axon_site/_ro/trn_rl_repo"):
    if os.path.isdir(_p) and _p not in sys.path:
        sys.path.insert(0, _p)

import numpy as np
import ml_dtypes

import concourse.bacc as bacc
import concourse.mybir as mybir
from concourse import tile
from concourse.bass_utils import run_bass_kernel_spmd

B, K, D = 2048, 100000, 50
N_CORES = 8
BC = B // N_CORES          # 256 batch rows per core
CHUNK = 128                # k rows per matmul chunk
GROUP = 16                 # chunks per DMA group
KP = 100352                # 49 * 2048, zero-padded K
NG = KP // (CHUNK * GROUP) # 49 DMA groups
NCHUNK = KP // CHUNK       # 784 chunks
DP1 = D + 1                # 51 (M columns + ones column)
MNB = 4                    # mn DMA block: groups per transfer.  Each DMA
                           # descriptor covers one partition's contiguous run;
                           # per-group mn transfers cost 128 descriptors for
                           # only 1.6KB each (~650us of total DMA queue time,
                           # saturating the engines) -- 4-group blocks move
                           # the same bytes in 1/4 the descriptors.
NB = (NG + MNB - 1) // MNB # 13 mn blocks (last one padded)
EXP_BIAS = -25.0

RING = 14                  # sT ring depth in chunks (14*256 f32 = 7 PSUM banks)
PT_REVS = 4                # pT ring depth in revolutions (4*14 chunks)
MM2_LAG = 20               # min chunks between mm1 and the consuming mm2
MM2_BATCH = 12             # mm2 chunks emitted per burst.  The first mm1
                           # after a burst pays an LDWEIGHTS-pipe backlog
                           # roughly proportional to burst length (mm2 LDW
                           # 45ns > mm2 MM 26ns), so smaller bursts dovetail
                           # better: measured +3.1us total mm1 stall at 12 vs
                           # +6.6us at 24.  lag+batch must stay under the pT
                           # ring depth (4*14=56).
# No dedicated warm-up burst: mm1 is now a full-array 128-row matmul, so the
# first ~3.4us of real chunks ramp the HAM clock themselves (at 1.2 GHz) and
# everything after runs at 2.4 GHz.  A warm-up burst would only delay the
# first chunk.
WARMUP_MM = 0

LOG2E = 1.4426950408889634
A16 = 128.0 * LOG2E                      # Schraudolph slope for bf16-bitcast
SIGMA = 0.05340                          # calibrated: zero mean log-error
B16 = 128.0 * (127.0 - SIGMA) + EXP_BIAS * A16
DR = D + 1                               # mt rows: D scaled-M rows + B16 row

# Per-revolution exp window grid: (start_col, end_col, engine).  Each window
# is its own PSUM tile (hazard tracking is tile-granular) so sizes must be
# whole PSUM banks (multiples of 512 f32): 2+2+1+2 banks here, acc is the
# 8th.  GPSIMD cannot read PSUM (BIR verifier), so ACT + DVE share the exp
# work; both are kept at <=80% so window reads finish with >100ns of margin
# (late window reads stall the PE past the clock-demotion threshold).
WINDOWS = [
    (0, 1024, "A"),
    (1024, 2048, "D"),
    (2048, 3072, "A"),
    (3072, 3584, "D"),
]

_nc_cache = None


def _install_trace_support():
    """The container's antenv lacks axon_hooks; synthesize it from trn_boot's
    ctypes NTFF shim so run_bass_kernel_spmd(trace=True) can profile."""
    import types

    if "antenv.axon_hooks" not in sys.modules:
        bootdir = "/root/.axon_site/trn_agent_boot"
        if bootdir not in sys.path:
            sys.path.insert(0, bootdir)
        import trn_boot

        hook = trn_boot._ntff_profile_via_ctypes("/opt/axon/libaxon_pjrt.so")
        mod = types.ModuleType("antenv.axon_hooks")
        mod.get_axon_ntff_profile_hook = lambda: hook
        mod.set_axon_ntff_profile_hook = lambda h: None
        sys.modules["antenv.axon_hooks"] = mod

    # No artifact bucket in this container; keep the NEFF dir local.
    import concourse.bass_utils as bu

    bu.upload_artifacts = lambda tmpdir: tmpdir


def _build():
    fp16 = mybir.dt.float16
    bf16 = mybir.dt.bfloat16
    f32 = mybir.dt.float32
    i16 = mybir.dt.int16

    nc = bacc.Bacc("TRN2", debug=False, num_devices=N_CORES)
    # xt is padded to 128 contraction rows (rows DR..127 zero) so mm1 can be a
    # FULL-ARRAY matmul: partial row-group MMs (50 rows) don't count as PE
    # activity for the HAM clock gate, which re-throttled the whole main loop
    # to 1.2 GHz in the previous version.  Full 128-row weights also enable
    # FWL (fast weight load), halving mm1 LDWEIGHTS to ~53ns.
    #
    # The Schraudolph exp affine (i16 = A16*s + B16) is folded INTO mm1: the
    # host pre-scales M by A16 and row D of the stationary holds B16, paired
    # with a ones-row in xt.  Both exp engines then run plain f32->i16
    # convert-copies (ACT: no spline table, cheaper op class).  The fp16
    # rounding of B16 is a GLOBAL factor on all exp values and cancels in the
    # softmax ratio.
    xt_d = nc.dram_tensor("xt", [CHUNK, BC], fp16, kind="ExternalInput")
    mtp_d = nc.dram_tensor("mtp", [DR, KP], fp16, kind="ExternalInput")
    mnp_d = nc.dram_tensor(
        "mnp", [NB, CHUNK, MNB * GROUP * DP1], bf16, kind="ExternalInput"
    )
    out_d = nc.dram_tensor("outU", [CHUNK, 2 * DP1], f32, kind="ExternalOutput")

    # window end -> (start, engine), keyed by the in-revolution chunk index
    # whose mm1 completes the window
    trig = {(we + BC - 1) // BC - 1: (ws, we, eng) for ws, we, eng in WINDOWS}

    with tile.TileContext(nc) as tc:
        with (
            tc.tile_pool(name="const", bufs=1) as constp,
            tc.tile_pool(name="mt", bufs=7) as mt_pool,
            tc.tile_pool(name="mn", bufs=5) as mn_pool,
            tc.tile_pool(name="wt0", bufs=PT_REVS) as wt0_pool,
            tc.tile_pool(name="wt1", bufs=PT_REVS) as wt1_pool,
            tc.tile_pool(name="wt2", bufs=PT_REVS) as wt2_pool,
            tc.tile_pool(name="wt3", bufs=PT_REVS) as wt3_pool,
            tc.tile_pool(name="wt4", bufs=PT_REVS) as wt4_pool,
            tc.tile_pool(name="ps0", bufs=1, space="PSUM") as ps0_pool,
            tc.tile_pool(name="ps1", bufs=1, space="PSUM") as ps1_pool,
            tc.tile_pool(name="ps2", bufs=1, space="PSUM") as ps2_pool,
            tc.tile_pool(name="ps3", bufs=1, space="PSUM") as ps3_pool,
            tc.tile_pool(name="ps4", bufs=1, space="PSUM") as ps4_pool,
            tc.tile_pool(name="acc", bufs=1, space="PSUM") as acc_pool,
        ):
            wpools = [wt0_pool, wt1_pool, wt2_pool, wt3_pool, wt4_pool]
            pspools = [ps0_pool, ps1_pool, ps2_pool, ps3_pool, ps4_pool]
            xt = constp.tile([CHUNK, BC], fp16)
            nc.sync.dma_start(out=xt[:], in_=xt_d[:])
            bias = constp.tile([CHUNK, 1], f32)
            nc.vector.memset(bias[:], EXP_BIAS)
            dummy = constp.tile([CHUNK, 1], bf16)
            # front-load any ACT table load for the Copy func during input DMA
            nc.scalar.copy(dummy[:].bitcast(i16), bias[:])

            # per-window sT tiles in PSUM (7 banks total), single-buffered:
            # the "ring" reuse hazard is at window granularity
            spsum = [
                pspools[i].tile([CHUNK, we - ws], f32, name=f"spsum{i}")
                for i, (ws, we, _) in enumerate(WINDOWS)
            ]
            acc = acc_pool.tile([CHUNK, 2 * DP1], f32)       # 1 PSUM bank
            # per-window pT tiles (separate tiles so the engines' writes
            # carry no false hazards against each other); keyed by window
            # index, double-buffered across revolutions by the pools
            wtiles = {}

            # Pre-zero rows D..127 of the mt ring slots once: the stationary
            # operand is read as full [128, .] tiles (for FWL + full-array HAM
            # activity) but the DMA only fills rows 0..D-1; rows D..127 must be
            # 0.0, not SBUF junk (junk can hold Inf/NaN patterns and 0*Inf =
            # NaN).  Each ring position gets its own tag with bufs=1 so every
            # allocation of that tag reuses the same physical slot — the
            # scheduler's lifetime-based slot assignment would otherwise let a
            # never-memset slot slip in.
            for i in range(7):
                mtz = mt_pool.tile(
                    [CHUNK, CHUNK * GROUP], fp16, name="mtz", tag=f"mt{i}", bufs=1
                )
                # full-tile memset on the otherwise-idle GpSimd engine (a
                # row-sliced memset would need a 32-aligned base partition;
                # the group DMA overwrites rows 0..D-1 anyway)
                nc.gpsimd.memset(mtz[:], 0.0)

            mt_tiles = {}
            mn_blocks = {}

            CG = CHUNK * GROUP
            GW = GROUP * DP1

            def issue_group_dma(g):
                if g >= NG:
                    return
                # Each dma_start costs ~700ns of DIRECT2D descriptor work on
                # the Sync sequencer, so steady-state groups use ONE transfer
                # per tensor (a 4-way split everywhere put 296 DIRECT2Ds =
                # 218us on it and starved the PE).  Only the early groups are
                # split so the kernel's first chunks land sooner.
                mt = mt_pool.tile(
                    [CHUNK, CG], fp16, name="mt", tag=f"mt{g % 7}", bufs=1
                )
                # alternate between the two HWDGE queues (sync=Q_I,
                # scalar=Q_X): the runtime serves each queue with its own set
                # of DMA engines, and Q_I alone is sometimes given only 3
                eng = nc.sync if g % 2 == 0 else nc.scalar
                # every group split 2-way (4-way for the startup-critical
                # first three): halves per-group landing latency when the DMA
                # fabric runs slow, at ~65 DIRECT2Ds per queue (well under
                # the ~300 that starved the sync sequencer)
                nsplit = 4 if g < 3 else 2
                for s in range(nsplit):
                    w = CG // nsplit
                    eng.dma_start(
                        out=mt[:DR, s * w : (s + 1) * w],
                        in_=mtp_d[:, g * CG + s * w : g * CG + (s + 1) * w],
                    )
                mt_tiles[g] = mt

            def issue_block_dma(b):
                if b >= NB:
                    return
                # issue from the ACT engine's HWDGE queue: all sync-issued
                # DMAs share one hardware queue (Q_I) whose engine assignment
                # varies run to run (155us vs 219us!); a second queue doubles
                # the DMA engines serving the kernel's ~21MB
                mnb = mn_pool.tile([CHUNK, MNB * GW], bf16, name="mnb")
                half = MNB * GW // 2
                eng = nc.scalar if b % 2 == 0 else nc.sync
                eng.dma_start(out=mnb[:, :half], in_=mnp_d[b][:, :half])
                eng.dma_start(out=mnb[:, half:], in_=mnp_d[b][:, half:])
                mn_blocks[b] = mnb

            for g in range(4):
                issue_group_dma(g)
            for b in range(3):
                issue_block_dma(b)

            # 128-col block index within a revolution -> window index
            win_of = [
                next(i for i, (ws, we, _) in enumerate(WINDOWS) if ws <= col < we)
                for col in range(0, RING * BC, CHUNK)
            ]

            def emit_mm2_batch(chunks):
                # acc[b, d'] += pT_half[k, b]^T @ [M|1]_chunk[k, d'] with the
                # exp output pT as the stationary operand.  The whole kernel
                # is ONE psum accumulation group in the acc bank; emitting the
                # batch as [all h0, then all h1] keeps region switches rare.
                for h in range(2):
                    for ch in chunks:
                        g, j = divmod(ch, GROUP)
                        bi, gi = divmod(g, MNB)
                        mnb = mn_blocks[bi]
                        par = (ch // RING) % PT_REVS
                        revcol = (ch % RING) * BC + h * CHUNK
                        wi = win_of[revcol // CHUNK]
                        off = revcol - WINDOWS[wi][0]
                        c0 = (gi * GROUP + j) * DP1
                        nc.tensor.matmul(
                            acc[:, h * DP1 : (h + 1) * DP1],
                            wtiles[par, wi][:, off : off + CHUNK],
                            mnb[:, c0 : c0 + DP1],
                            start=(ch == 0 and h == 0),
                            stop=(ch == NCHUNK - 1 and h == 1),
                        )
                for ch in chunks:
                    g, j = divmod(ch, GROUP)
                    if j == GROUP - 1 and (g % MNB == MNB - 1 or g == NG - 1):
                        mn_blocks.pop(g // MNB)

            next_mm2 = 0
            for c in range(NCHUNK):
                g, j = divmod(c, GROUP)
                if j == 0:
                    issue_group_dma(g + 4)
                if c % (MNB * GROUP) == 0:
                    issue_block_dma(c // (MNB * GROUP) + 3)
                rev, rc = divmod(c, RING)
                # Flush the available mm2 backlog right BEFORE each
                # group-boundary mm1: that mm1 is the one that stalls when its
                # group's mt DMA is late, and the PE queue is FIFO -- ready
                # mm2 work queued behind it would stall too.  Emitted ahead of
                # it, ~1us of mm2 runs while the group lands.
                if j == 0 and c - MM2_LAG > next_mm2:
                    emit_mm2_batch(range(next_mm2, c - MM2_LAG))
                    next_mm2 = c - MM2_LAG
                revcol = rc * BC
                mwi = win_of[revcol // CHUNK]
                moff = revcol - WINDOWS[mwi][0]
                nc.tensor.matmul(
                    spsum[mwi][:, moff : moff + BC],
                    mt_tiles[g][:, j * CHUNK : (j + 1) * CHUNK],
                    xt[:],
                    start=True,
                    stop=True,
                )
                if j == GROUP - 1:
                    mt_tiles.pop(g)
                if rc in trig:
                    ws, we, eng = trig[rc]
                    wi = WINDOWS.index((ws, we, eng))
                    wt = wpools[wi].tile([CHUNK, we - ws], bf16, name=f"wt{wi}")
                    wtiles[rev % PT_REVS, wi] = wt
                    src = spsum[wi][:]
                    # mm1 already produced A16*s + B16; the exp is now just a
                    # rounding f32->i16 convert whose bitcast-to-bf16 is
                    # exp(s-25) to within the usual Schraudolph +-2%
                    if eng == "A":
                        nc.scalar.copy(wt[:].bitcast(i16), src)
                    else:
                        nc.vector.tensor_copy(wt[:].bitcast(i16), src)

            while next_mm2 < NCHUNK:
                hi = min(next_mm2 + MM2_BATCH, NCHUNK)
                emit_mm2_batch(range(next_mm2, hi))
                next_mm2 = hi

            out_sb = constp.tile([CHUNK, 2 * DP1], f32)
            nc.vector.tensor_copy(out_sb[:], acc[:])
            nc.sync.dma_start(out=out_d[:], in_=out_sb[:])

    nc.compile()
    return nc


def _get_nc():
    global _nc_cache
    if _nc_cache is None:
        _nc_cache = _build()
    return _nc_cache


def _prep_inputs(x, M):
    x = np.asarray(x, dtype=np.float32)
    M = np.asarray(M, dtype=np.float32)

    mtp = np.zeros((DR, KP), dtype=np.float16)
    mtp[:D, :K] = (A16 * M.T).astype(np.float16)
    mtp[D, :] = np.float16(B16)

    mn = np.zeros((KP, DP1), dtype=np.float32)
    mn[:K, :D] = M
    mn[:, D] = 1.0
    # [g, j, p, d] -> [g, p, j*51+d] so each partition's row is contiguous
    mnp1 = np.ascontiguousarray(
        mn.reshape(NG, GROUP, CHUNK, DP1).transpose(0, 2, 1, 3)
    ).reshape(NG, CHUNK, GROUP * DP1)
    # pack MNB groups per DMA block: [b, p, gi*816 + j*51 + d], one contiguous
    # per-partition run per block (4x fewer DMA descriptors)
    mnp_pad = np.zeros((NB * MNB, CHUNK, GROUP * DP1), dtype=np.float32)
    mnp_pad[:NG] = mnp1
    mnp = np.ascontiguousarray(
        mnp_pad.reshape(NB, MNB, CHUNK, GROUP * DP1).transpose(0, 2, 1, 3)
    ).reshape(NB, CHUNK, MNB * GROUP * DP1).astype(ml_dtypes.bfloat16)

    in_maps = []
    for i in range(N_CORES):
        xt = np.zeros((CHUNK, BC), dtype=np.float16)
        xt[:D] = x[i * BC : (i + 1) * BC].T.astype(np.float16)
        xt[D] = np.float16(1.0)  # pairs with the B16 row of the stationary
        in_maps.append({"xt": xt, "mtp": mtp, "mnp": mnp})
    return in_maps


def _run(x, M, trace=False):
    if trace:
        _install_trace_support()
    nc = _get_nc()
    in_maps = _prep_inputs(x, M)
    res = run_bass_kernel_spmd(nc, in_maps, core_ids=list(range(N_CORES)), trace=trace)
    x = np.asarray(x, dtype=np.float32)
    u = np.empty((B, D), dtype=np.float32)
    for i in range(N_CORES):
        raw = res.results[i]["outU"]  # [128, 2*51] — per-half accumulators
        for h in range(2):
            seg = raw[:, h * DP1 : (h + 1) * DP1]  # [128, 51] natural [b, d']
            r0 = i * BC + h * CHUNK
            u[r0 : r0 + CHUNK] = seg[:, :D] / seg[:, D : D + 1]
    out = np.concatenate([x, u], axis=1)
    return out, res


def kernel(x, M):
    out, _ = _run(x, M, trace=False)
    return out

